# revision 14
# baseline (speedup 1.0000x reference)
"""DAN (FISTA sparse coding) kernel for 8 TRN2 NeuronCores — D-form.

Math (per reference):
  D = real_D(rho, theta)                   (36, 161)  host-computed
  A = I - L_inv * D^T D;  B = L_inv D^T Y
  100 FISTA iterations, restructured through the rank-36 residual:
    r_k = Y - D @ y_k                      -> PE mm1: [-D^T; I]-stack @ [y; Y]
    m_k = y_k + L_inv D^T r_k  (= A y_k + B)
         -> PE mm2: K=36 matmuls, row-packed 2x via tile_position
         -> the "+ y_k" rides the custom DVE op below
    x_{k+1} = softshrink(m_k, thr)         -> custom DVE op SOFTSHRINK_ADD
    y_{k+1} = (1+a_k) x_{k+1} - a_k x_k    -> GPSIMD (rows 0:128) + DVE ln_bwd_dx
  C = x_100; reconst = D @ C

Sharding: data-parallel over batch, 32 batch elems (bd=1600 cols) per core.
r is placed at PSUM partitions 0:36 (cols 0:800) and 64:100 (cols 800:1600)
via matmul col-placement so mm2 can run both halves concurrently in
disjoint PE row-groups.
"""

import numpy as np

T = 36
P = 161          # 2*80+1 atoms
LAM = np.float32(0.1)
MAX_ITER = 100
BATCH = 256
DDIM = 50
NCORES = 8
BPC = BATCH // NCORES          # 32 batch elems per core
BD = BPC * DDIM                # 1600 columns per core
CH = 400                       # matmul chunk (fp32 moving max is 512)
NCH = BD // CH                 # 4 chunks
K2 = (P - 128) + T             # 69: w-tail rows + data rows (mm1 K-tile 2)


def _host_prep(rho, theta):
    """Replicate reference.real_D and FISTA constants in fp32 numpy."""
    rho = rho.astype(np.float32)
    theta = theta.astype(np.float32)
    rmin, rmax = np.float32(0.001), np.float32(1.15)
    r = rmin + (rmax - rmin) / (np.float32(1.0) + np.exp(-rho))
    th = np.float32(np.pi) / (np.float32(1.0) + np.exp(-theta))
    i = np.arange(T, dtype=np.float32)[:, None]
    p = r[None, :] ** i
    W1 = p * np.cos(i * th[None, :])
    W2 = p * np.sin(i * th[None, :])
    D = np.concatenate([np.ones((T, 1), np.float32), W1, W2], axis=1)
    norms = np.linalg.norm(D[:, 1:], axis=0, keepdims=True)
    D = np.concatenate([D[:, :1], D[:, 1:] / norms], axis=1).astype(np.float32)

    DtD = (D.T @ D).astype(np.float32)
    L = np.linalg.norm(DtD, ord=2).astype(np.float32)
    L_inv = np.float32(1.0) / L
    thr = LAM * L_inv

    t = np.float32(1.0)
    tts = []
    for _ in range(MAX_ITER):
        t_new = (np.float32(1.0)
                 + np.sqrt(np.float32(1.0) + np.float32(4.0) * t * t)) / np.float32(2.0)
        t_new = np.float32(t_new)
        tts.append(np.float32((t - np.float32(1.0)) / t_new))
        t = t_new
    return D, L_inv, thr, tts


def _register_ss_add():
    """out = softshrink(in0 + in1, s0) as one DVE pass."""
    import concourse.dve_ops as dve_ops
    from concourse.dve_spec import Spec, Src0, Src1, C0, Zero, maxx, minn, lower
    from concourse.dve_uop import DveOpSpec

    name = "SOFTSHRINK_ADD_ANT"
    for op in dve_ops.OPS:
        if op.name == name:
            return op
    z = Src0 + Src1
    body = maxx(z - C0, Zero) + minn(z + C0, Zero)

    def ref(in0, in1, s0, s1, imm2):
        zz = in0.astype(np.float32) + in1.astype(np.float32)
        return (np.maximum(zz - s0, 0) + np.minimum(zz + s0, 0)).astype(np.float32)

    spec = Spec(body=body, reference=ref)
    row = dve_ops._CUSTOM_DVE_ROW_BASE + len(dve_ops.OPS)
    assert row < 0x20
    dve_ops._SUB_OPCODE_FOR_NAME[name] = row
    shas = {}
    for ver in ("v3", "v4"):
        try:
            s = DveOpSpec(name=name, opcode=row, uops=lower(spec, ver=ver),
                          rd1_en=True)
            shas[ver] = s.sha(ver)
        except Exception:
            pass
    op = dve_ops.DveOp(name, spec, subdim=False, uops_sha=shas)
    dve_ops.OPS.append(op)
    dve_ops.CUSTOM_DVE_SPECS[name] = spec
    return op


def _build_kernel(thr, tts):
    import concourse.bacc as bacc
    import concourse.mybir as mybir
    import concourse.tile as tile

    f32 = mybir.dt.float32
    ALU = mybir.AluOpType
    SS = _register_ss_add()

    nc = bacc.Bacc("TRN2", target_bir_lowering=False, debug=False,
                   num_devices=NCORES)

    d_xs = nc.dram_tensor("xs", [BPC, T, DDIM], f32, kind="ExternalInput")
    d_nst1 = nc.dram_tensor("nst1", [128, T], f32, kind="ExternalInput")
    d_nst2 = nc.dram_tensor("nst2", [K2, T], f32, kind="ExternalInput")
    d_dstat = nc.dram_tensor("dstat", [100, P], f32, kind="ExternalInput")
    d_dt1 = nc.dram_tensor("dt1", [128, T], f32, kind="ExternalInput")
    d_dt2 = nc.dram_tensor("dt2", [P - 128, T], f32, kind="ExternalInput")
    d_c = nc.dram_tensor("c_out", [BPC, P, DDIM], f32, kind="ExternalOutput")
    d_rec = nc.dram_tensor("rec_out", [BPC, T, DDIM], f32, kind="ExternalOutput")

    with tile.TileContext(nc) as tc:
        with (
            tc.tile_pool(name="cst", bufs=1) as cst,
            tc.tile_pool(name="tmp", bufs=3) as tmp,
            tc.tile_pool(name="ps", bufs=1, space="PSUM") as ps,
        ):
            nst1 = cst.tile([128, T], f32)
            nst2 = cst.tile([K2, T], f32)
            dstat = cst.tile([100, P], f32)
            dt1 = cst.tile([128, T], f32)
            dt2 = cst.tile([P - 128, T], f32)
            nc.sync.dma_start(nst1[:], d_nst1.ap()[:, :])
            nc.sync.dma_start(nst2[:], d_nst2.ap()[:, :])
            nc.sync.dma_start(dstat[:], d_dstat.ap()[:, :])
            nc.sync.dma_start(dt1[:], d_dt1.ap()[:, :])
            nc.sync.dma_start(dt2[:], d_dt2.ap()[:, :])

            # State: y_k rows 0:128 in w1; rows 128:161 in v2[0:33];
            # v2[33:69] = Y (data, (t, b*d) layout).
            w1 = cst.tile([128, BD], f32)
            v2 = cst.tile([K2, BD], f32)
            xa1 = cst.tile([128, BD], f32)
            xa2 = cst.tile([P - 128, BD], f32)
            xb1 = cst.tile([128, BD], f32)
            xb2 = cst.tile([P - 128, BD], f32)
            # residual staging: halves at partitions 0:36 / 64:100
            rsb = cst.tile([100, 2 * CH], f32)

            nc.sync.dma_start(
                v2[P - 128:K2, :].rearrange("t (b j) -> t b j", b=BPC),
                d_xs.ap().rearrange("b t j -> t b j"))
            nc.vector.memset(w1[:], 0.0)
            nc.vector.memset(v2[0:P - 128, :], 0.0)
            nc.vector.memset(xb1[:], 0.0)
            nc.vector.memset(xb2[:], 0.0)

            thrf = float(thr)
            for k in range(MAX_ITER):
                a_k = float(tts[k])
                b_k = a_k / (1.0 + a_k)
                xn1, xn2 = (xa1, xa2) if k % 2 == 0 else (xb1, xb2)
                xo1, xo2 = (xb1, xb2) if k % 2 == 0 else (xa1, xa2)

                # mm1: r = Y - D @ y  (chunk order 0,2,1,3 so each
                # off-pair completes early for its ACT copy)
                rps = ps.tile([100, 1024], f32, tag="rps")
                for c in (0, 2, 1, 3):
                    pb = 64 * (c // 2)
                    off = (c % 2) * 512
                    c0 = c * CH
                    tp = (0, pb) if pb else None
                    nc.tensor.matmul(rps[pb:pb + T, off:off + CH], nst1[:],
                                     w1[:, c0:c0 + CH],
                                     start=True, stop=False, tile_position=tp)
                    nc.tensor.matmul(rps[pb:pb + T, off:off + CH], nst2[:],
                                     v2[:, c0:c0 + CH],
                                     start=False, stop=True, tile_position=tp)

                # r psum -> sbuf (one ACT copy per bank covers both halves)
                for cp in range(2):
                    nc.scalar.copy(rsb[:, cp * CH:(cp + 1) * CH],
                                   rps[:, cp * 512:cp * 512 + CH])

                # mm2 + softshrink-add per chunk
                for dc in range(2):
                    for j in range(2):
                        c = dc * 2 + j
                        pb = 64 * (c // 2)
                        cp = (c % 2) * CH
                        c0 = c * CH
                        tp = (pb, 0) if pb else None
                        pm1 = ps.tile([128, 512], f32, tag="pm1", bufs=3)
                        pm2 = ps.tile([P - 128, 512], f32, tag="pm2", bufs=3)
                        nc.tensor.matmul(pm1[:, 0:CH],
                                         dstat[pb:pb + T, 0:128],
                                         rsb[pb:pb + T, cp:cp + CH],
                                         start=True, stop=True,
                                         tile_position=tp)
                        nc.tensor.matmul(pm2[:, 0:CH],
                                         dstat[pb:pb + T, 128:P],
                                         rsb[pb:pb + T, cp:cp + CH],
                                         start=True, stop=True,
                                         tile_position=tp)
                        nc.vector._custom_dve(
                            SS, out=xn1[:, c0:c0 + CH], in0=pm1[:, 0:CH],
                            in1=w1[:, c0:c0 + CH], s0=thrf)
                        nc.vector._custom_dve(
                            SS, out=xn2[:, c0:c0 + CH], in0=pm2[:, 0:CH],
                            in1=v2[0:P - 128, c0:c0 + CH], s0=thrf)

                    if k < MAX_ITER - 1:
                        sl = slice(dc * 2 * CH, (dc + 1) * 2 * CH)
                        nc.vector.ln_bwd_dx(
                            out=w1[:, sl], dy=xn1[:, sl], x_hat=xo1[:, sl],
                            mean_dyx=b_k, mean_dy=0.0, scale=1.0 + a_k)
                        nc.vector.ln_bwd_dx(
                            out=v2[0:P - 128, sl], dy=xn2[:, sl],
                            x_hat=xo2[:, sl],
                            mean_dyx=b_k, mean_dy=0.0, scale=1.0 + a_k)

            # ---- epilogue: C out + reconst = D @ C ----
            xf1, xf2 = (xa1, xa2) if (MAX_ITER - 1) % 2 == 0 else (xb1, xb2)
            c_ap = d_c.ap().rearrange("b p j -> p b j")
            nc.sync.dma_start(c_ap[0:128],
                              xf1.rearrange("p (b j) -> p b j", b=BPC))
            nc.sync.dma_start(c_ap[128:P],
                              xf2.rearrange("p (b j) -> p b j", b=BPC))

            rec = cst.tile([T, BD], f32)
            for dc in range(2):
                pr = ps.tile([T, 1024], f32, tag="rps")
                for j in range(2):
                    c0 = (dc * 2 + j) * CH
                    o0 = j * 512
                    nc.tensor.matmul(pr[:, o0:o0 + CH], dt1[:],
                                     xf1[:, c0:c0 + CH], start=True, stop=False)
                    nc.tensor.matmul(pr[:, o0:o0 + CH], dt2[:],
                                     xf2[:, c0:c0 + CH], start=False, stop=True)
                prv = pr.rearrange("p (b f) -> p b f", f=512)[:, :, 0:CH]
                s0 = dc * 2 * CH
                recv = rec[:, s0:s0 + 2 * CH].rearrange(
                    "p (b f) -> p b f", f=CH)
                nc.vector.tensor_copy(recv, prv)
            nc.sync.dma_start(
                d_rec.ap().rearrange("b t j -> t b j"),
                rec.rearrange("t (b j) -> t b j", b=BPC))

    nc.compile()
    return nc


def kernel(x, rho, theta):
    from concourse.bass_utils import run_bass_kernel_spmd

    x = np.ascontiguousarray(x, dtype=np.float32)
    D, L_inv, thr, tts = _host_prep(rho, theta)

    Dt = D.T.astype(np.float32)                       # (161, 36)
    nst1 = np.ascontiguousarray(-Dt[0:128, :])
    nst2 = np.concatenate([-Dt[128:P, :], np.eye(T, dtype=np.float32)],
                          axis=0)
    dstat = np.zeros((100, P), np.float32)
    dstat[0:T] = L_inv * D
    dstat[64:64 + T] = L_inv * D
    dt1 = np.ascontiguousarray(Dt[0:128, :])
    dt2 = np.ascontiguousarray(Dt[128:P, :])

    nc = _build_kernel(thr, tts)

    in_maps = []
    for i in range(NCORES):
        in_maps.append({
            "xs": np.ascontiguousarray(x[i * BPC:(i + 1) * BPC]),
            "nst1": nst1, "nst2": np.ascontiguousarray(nst2),
            "dstat": dstat, "dt1": dt1, "dt2": dt2,
        })
    res = run_bass_kernel_spmd(nc, in_maps, core_ids=list(range(NCORES)))

    C = np.concatenate([r["c_out"] for r in res.results], axis=0)
    rec = np.concatenate([r["rec_out"] for r in res.results], axis=0)
    return C, D, rec


# revision 17
# speedup vs baseline: 1.3342x; 1.3342x over previous
"""DAN (FISTA sparse coding) kernel for 8 TRN2 NeuronCores.

Math (per reference):
  D = real_D(rho, theta)                  (36, 161)  host-computed
  A = I - L_inv * D^T D                   (161, 161)
  B = L_inv * D^T Y                       (161, bd)  folded into matmul
  100 FISTA iterations:
    m_k    = A @ y_k + B                  -> PE: [A; L_inv*D]^T-stack @ [y; Y]
    x_{k+1} = softshrink(m_k, thr)        -> ACT relu(m-thr), relu(-m-thr); DVE sub
    y_{k+1} = (1+a_k) x_{k+1} - a_k x_k   -> DVE custom op ln_bwd_dx (1 op)
  C = x_100; reconst = D @ C

Sharding: data-parallel over batch, 32 batch elems (bd=1600 cols) per core.
Layout on device: (p=161, batchd) split as rows [0:128] + tail [128:161];
the K2 stationary tile stacks A-tail rows with L_inv*D so the B-add rides
the otherwise-wasted PE contraction rows.
"""

import numpy as np

T = 36
P = 161          # 2*80+1 atoms
LAM = np.float32(0.1)
MAX_ITER = 100
BATCH = 256
DDIM = 50
NCORES = 8
BPC = BATCH // NCORES          # 32 batch elems per core
BD = BPC * DDIM                # 1600 columns per core
CH = 400                       # matmul chunk (fp32 moving max is 512)
NCH = BD // CH                 # 4 chunks
DC = 2                         # chunks per elementwise group
K2 = (P - 128) + T             # 69: A-tail rows + dictionary rows
DEBUG_W = False                # dump W state after each iteration
TRACE = False                  # capture NTFF profile (needs axon hook shim)
LAST_RESULT = None             # BassKernelResults of the last kernel() call


def _host_prep(rho, theta):
    """Replicate reference.real_D and FISTA constants in fp32 numpy."""
    rho = rho.astype(np.float32)
    theta = theta.astype(np.float32)
    rmin, rmax = np.float32(0.001), np.float32(1.15)
    r = rmin + (rmax - rmin) / (np.float32(1.0) + np.exp(-rho))
    th = np.float32(np.pi) / (np.float32(1.0) + np.exp(-theta))
    i = np.arange(T, dtype=np.float32)[:, None]
    p = r[None, :] ** i
    W1 = p * np.cos(i * th[None, :])
    W2 = p * np.sin(i * th[None, :])
    D = np.concatenate([np.ones((T, 1), np.float32), W1, W2], axis=1)
    norms = np.linalg.norm(D[:, 1:], axis=0, keepdims=True)
    D = np.concatenate([D[:, :1], D[:, 1:] / norms], axis=1).astype(np.float32)

    DtD = (D.T @ D).astype(np.float32)
    L = np.linalg.norm(DtD, ord=2).astype(np.float32)
    L_inv = np.float32(1.0) / L
    thr = LAM * L_inv
    A = (np.eye(P, dtype=np.float32) - L_inv * DtD).astype(np.float32)

    t = np.float32(1.0)
    tts = []
    for _ in range(MAX_ITER):
        t_new = (np.float32(1.0)
                 + np.sqrt(np.float32(1.0) + np.float32(4.0) * t * t)) / np.float32(2.0)
        t_new = np.float32(t_new)
        tts.append(np.float32((t - np.float32(1.0)) / t_new))
        t = t_new
    return D, A, L_inv, thr, tts


def _build_kernel(thr, tts):
    import concourse.bacc as bacc
    import concourse.mybir as mybir
    import concourse.tile as tile

    f32 = mybir.dt.float32
    Relu = mybir.ActivationFunctionType.Relu

    nc = bacc.Bacc("TRN2", target_bir_lowering=False, debug=False,
                   num_devices=NCORES)

    d_xs = nc.dram_tensor("xs", [BPC, T, DDIM], f32, kind="ExternalInput")
    d_st1 = nc.dram_tensor("st1", [128, P], f32, kind="ExternalInput")
    d_st2 = nc.dram_tensor("st2", [K2, P], f32, kind="ExternalInput")
    d_dt1 = nc.dram_tensor("dt1", [128, T], f32, kind="ExternalInput")
    d_dt2 = nc.dram_tensor("dt2", [P - 128, T], f32, kind="ExternalInput")
    d_c = nc.dram_tensor("c_out", [BPC, P, DDIM], f32, kind="ExternalOutput")
    d_rec = nc.dram_tensor("rec_out", [BPC, T, DDIM], f32, kind="ExternalOutput")
    d_wdbg = None
    if DEBUG_W:
        d_wdbg = nc.dram_tensor("w_dbg", [MAX_ITER, P, BD], f32,
                                kind="ExternalOutput")
        d_xdbg = nc.dram_tensor("x_dbg", [MAX_ITER, P, BD], f32,
                                kind="ExternalOutput")

    with tile.TileContext(nc) as tc:
        with (
            tc.tile_pool(name="cst", bufs=1) as cst,
            tc.tile_pool(name="tmp", bufs=3) as tmp,
            tc.tile_pool(name="ps", bufs=2, space="PSUM") as ps,
        ):
            st1 = cst.tile([128, P], f32)
            st2 = cst.tile([K2, P], f32)
            dt1 = cst.tile([128, T], f32)
            dt2 = cst.tile([P - 128, T], f32)
            nc.sync.dma_start(st1[:], d_st1.ap()[:, :])
            nc.sync.dma_start(st2[:], d_st2.ap()[:, :])
            nc.sync.dma_start(dt1[:], d_dt1.ap()[:, :])
            nc.sync.dma_start(dt2[:], d_dt2.ap()[:, :])

            # State: W (=y_k) rows 0:128 in w1; rows 128:161 live in v2[0:33];
            # v2[33:69] holds Y (the data, in (t, b*d) layout).
            w1 = cst.tile([128, BD], f32)
            v2 = cst.tile([K2, BD], f32)
            xa1 = cst.tile([128, BD], f32)
            xa2 = cst.tile([P - 128, BD], f32)
            xb1 = cst.tile([128, BD], f32)
            xb2 = cst.tile([P - 128, BD], f32)

            nthr = cst.tile([128, 1], f32)
            nc.vector.memset(nthr[:], -float(thr))

            nc.sync.dma_start(
                v2[P - 128:K2, :].rearrange("t (b j) -> t b j", b=BPC),
                d_xs.ap().rearrange("b t j -> t b j"))
            nc.vector.memset(w1[:], 0.0)
            nc.vector.memset(v2[0:P - 128, :], 0.0)
            nc.vector.memset(xb1[:], 0.0)
            nc.vector.memset(xb2[:], 0.0)

            for k in range(MAX_ITER):
                a_k = float(tts[k])
                xn1, xn2 = (xa1, xa2) if k % 2 == 0 else (xb1, xb2)
                xo1, xo2 = (xb1, xb2) if k % 2 == 0 else (xa1, xa2)
                for dc in range(NCH // DC):
                    pm1 = ps.tile([128, 1024], f32, tag="pm1")
                    pm2 = ps.tile([P - 128, 1024], f32, tag="pm2")
                    for j in range(DC):
                        c0 = (dc * DC + j) * CH
                        o0 = j * 512
                        mv1 = w1[:, c0:c0 + CH]
                        mv2 = v2[:, c0:c0 + CH]
                        nc.tensor.matmul(pm1[:, o0:o0 + CH], st1[:, 0:128],
                                         mv1, start=True, stop=False)
                        nc.tensor.matmul(pm1[:, o0:o0 + CH], st2[:, 0:128],
                                         mv2, start=False, stop=True)
                        nc.tensor.matmul(pm2[:, o0:o0 + CH], st1[:, 128:P],
                                         mv1, start=True, stop=False)
                        nc.tensor.matmul(pm2[:, o0:o0 + CH], st2[:, 128:P],
                                         mv2, start=False, stop=True)

                    # elementwise over this 800-col group
                    s0 = dc * DC * CH
                    pm1v = pm1.rearrange("p (b f) -> p b f", f=512)[:, :, 0:CH]
                    pm2v = pm2.rearrange("p (b f) -> p b f", f=512)[:, :, 0:CH]
                    pt1 = tmp.tile([128, DC, CH], f32, tag="pt1")
                    qt1 = tmp.tile([128, DC, CH], f32, tag="qt1")
                    pt2 = tmp.tile([P - 128, DC, CH], f32, tag="pt2")
                    qt2 = tmp.tile([P - 128, DC, CH], f32, tag="qt2")
                    nc.scalar.activation(pt1[:], pm1v, Relu, bias=nthr[:])
                    nc.scalar.activation(qt1[:], pm1v, Relu, bias=nthr[:],
                                         scale=-1.0)
                    nc.scalar.activation(pt2[:], pm2v, Relu, bias=nthr[0:P - 128])
                    nc.scalar.activation(qt2[:], pm2v, Relu, bias=nthr[0:P - 128],
                                         scale=-1.0)
                    xn1v = xn1[:, s0:s0 + DC * CH].rearrange(
                        "p (b f) -> p b f", f=CH)
                    xn2v = xn2[:, s0:s0 + DC * CH].rearrange(
                        "p (b f) -> p b f", f=CH)
                    nc.vector.tensor_sub(xn1v, pt1[:], qt1[:])
                    nc.vector.tensor_sub(xn2v, pt2[:], qt2[:])

                    if k < MAX_ITER - 1:
                        sl = slice(s0, s0 + DC * CH)
                        b_k = a_k / (1.0 + a_k)
                        nc.vector.ln_bwd_dx(
                            out=w1[:, sl], dy=xn1[:, sl], x_hat=xo1[:, sl],
                            mean_dyx=b_k, mean_dy=0.0, scale=1.0 + a_k)
                        nc.vector.ln_bwd_dx(
                            out=v2[0:P - 128, sl], dy=xn2[:, sl],
                            x_hat=xo2[:, sl],
                            mean_dyx=b_k, mean_dy=0.0, scale=1.0 + a_k)

                if DEBUG_W:
                    nc.sync.dma_start(d_wdbg.ap()[k, 0:128, :], w1[:])
                    nc.sync.dma_start(d_wdbg.ap()[k, 128:P, :],
                                      v2[0:P - 128, :])
                    nc.sync.dma_start(d_xdbg.ap()[k, 0:128, :], xn1[:])
                    nc.sync.dma_start(d_xdbg.ap()[k, 128:P, :], xn2[:])

            # ---- epilogue: C out + reconst = D @ C ----
            xf1, xf2 = (xa1, xa2) if (MAX_ITER - 1) % 2 == 0 else (xb1, xb2)
            c_ap = d_c.ap().rearrange("b p j -> p b j")
            nc.sync.dma_start(c_ap[0:128],
                              xf1.rearrange("p (b j) -> p b j", b=BPC))
            nc.sync.dma_start(c_ap[128:P],
                              xf2.rearrange("p (b j) -> p b j", b=BPC))

            rec = cst.tile([T, BD], f32)
            for dc in range(NCH // DC):
                pr = ps.tile([T, 1024], f32, tag="pm1")
                for j in range(DC):
                    c0 = (dc * DC + j) * CH
                    o0 = j * 512
                    nc.tensor.matmul(pr[:, o0:o0 + CH], dt1[:],
                                     xf1[:, c0:c0 + CH], start=True, stop=False)
                    nc.tensor.matmul(pr[:, o0:o0 + CH], dt2[:],
                                     xf2[:, c0:c0 + CH], start=False, stop=True)
                prv = pr.rearrange("p (b f) -> p b f", f=512)[:, :, 0:CH]
                s0 = dc * DC * CH
                recv = rec[:, s0:s0 + DC * CH].rearrange(
                    "p (b f) -> p b f", f=CH)
                nc.vector.tensor_copy(recv, prv)
            nc.sync.dma_start(
                d_rec.ap().rearrange("b t j -> t b j"),
                rec.rearrange("t (b j) -> t b j", b=BPC))

    nc.compile()
    return nc


def kernel(x, rho, theta):
    from concourse.bass_utils import run_bass_kernel_spmd

    x = np.ascontiguousarray(x, dtype=np.float32)
    D, A, L_inv, thr, tts = _host_prep(rho, theta)

    st1 = np.ascontiguousarray(A[0:128, :])
    st2 = np.ascontiguousarray(
        np.concatenate([A[128:P, :], L_inv * D], axis=0))
    dt1 = np.ascontiguousarray(D.T[0:128, :])
    dt2 = np.ascontiguousarray(D.T[128:P, :])

    nc = _build_kernel(thr, tts)

    in_maps = []
    for i in range(NCORES):
        in_maps.append({
            "xs": np.ascontiguousarray(x[i * BPC:(i + 1) * BPC]),
            "st1": st1, "st2": st2, "dt1": dt1, "dt2": dt2,
        })
    kwargs = {}
    if TRACE:
        kwargs = dict(trace=True, tmpdir="/tmp/kernel_ntff")
    res = run_bass_kernel_spmd(nc, in_maps, core_ids=list(range(NCORES)),
                               **kwargs)
    global LAST_RESULT
    LAST_RESULT = res

    C = np.concatenate([r["c_out"] for r in res.results], axis=0)
    rec = np.concatenate([r["rec_out"] for r in res.results], axis=0)
    return C, D, rec
